# revision 33
# baseline (speedup 1.0000x reference)
"""AttentionBlock (GroupNorm + single-head spatial attention + proj + residual)
on 8 trn2 NeuronCores, data-parallel over the batch (1 image per core).

fp8 build v2: same math as v1 (A = W_k^T W_q folded, x16 host scaling,
E = exp(logits - 1.5)), restructured around the measured v1 trace:
  - input DMA: x streams FIRST on both hardware DGE queues (sync + scalar),
    split into 8 half-ct transfers so bn_stats pipelines with arrival;
    weights follow x in queue order, tiny constants ride the gpsimd queue.
  - GroupNorm fully per-ct pipelined: stats for slot0 on Act (Identity/Square
    with accum_out), slots 1-3 on DVE; the per-group chain (var, rstd via
    pow(var+eps, -0.5), -mean*rstd) runs on the otherwise-idle GpSimd, with
    per-ct gsum/broadcast matmuls, so the first qkv GEMM starts ~6us earlier.
  - proj for pixel-half 0 is issued before attention of half 1, so the output
    DMA streams during the endgame instead of bunching after the last matmul.
  - proj evacuation is a DVE scalar_tensor_tensor (psum + pb + x residual),
    dropping v1's 8 residual identity matmuls and the bf16 x copy.
Measured v1: 60741ns; rel err ~5.4e-3 (budget 2e-2).
"""

import sys

sys.path.insert(0, "/opt/trn_rl_repo")

import numpy as np
import ml_dtypes

import concourse.bass as bass
import concourse.tile as tile
from concourse import bacc, mybir
from concourse.bass_utils import run_bass_kernel_spmd
from concourse.tile_rust import add_dep_helper

F32 = mybir.dt.float32
F32R = mybir.dt.float32r
BF16 = mybir.dt.bfloat16
FP8 = mybir.dt.float8e4
DR = mybir.MatmulPerfMode.DoubleRow
AOP = mybir.AluOpType
AF = mybir.ActivationFunctionType

C = 512          # channels
NPIX = 1024      # pixels per image (32*32)
CT = 4           # channel tiles of 128
JT = 8           # pixel tiles of 128
NH = 2           # halves of NPIX for the 512-wide moving dim
G = 32           # groups
GS = 16          # channels per group
EPS = 1e-5
SCALE = C ** -0.5
WSCALE = 16.0    # host-side scale on A and W_v (and the den ones-vector)
ESHIFT = -1.5    # softmax-invariant logit shift keeping exp() in fp8 range
WARM_MMS = 20    # PE warm-up matmuls during the input-DMA window
FILL_A = 8       # fillers bridging gsum/bc to the first t matmuls
FILL_B = 6       # fillers bridging t ci0 -> ci1
TRACE = False          # set True (from test.py) to capture an NTFF profile
TRACE_KW = {}          # extra kwargs for run_bass_kernel_spmd
LAST_RESULTS = None    # BassKernelResults of the most recent run

_cache = {}

# xn slices computed on Act (need the negated bias column); rest on DVE
XN_ACT = (0, 1)


def _build_fp8():
    nc = bacc.Bacc("TRN2")

    x_d = nc.dram_tensor("x", [128, CT, NPIX], F32, kind="ExternalInput")
    qa_d = nc.dram_tensor("qa", [128, CT, C], FP8, kind="ExternalInput")
    qv_d = nc.dram_tensor("qv", [128, CT, C], FP8, kind="ExternalInput")
    pw_d = nc.dram_tensor("pw", [128, CT, C], FP8, kind="ExternalInput")
    pb_d = nc.dram_tensor("pb", [128, CT], F32, kind="ExternalInput")
    y_d = nc.dram_tensor("y", [128, CT, NPIX], F32, kind="ExternalOutput")

    # Indicator constants for the cross-partition group reductions.
    # ind1 carries the 1/GS mean factor so the gsum psum is already [mean,
    # E[x^2]] per group; ind2 broadcasts group rows back to channels.
    ind1 = np.zeros((128, CT * G), np.float32)
    for ct in range(CT):
        for p in range(128):
            ind1[p, ct * G + ct * 8 + p // GS] = 1.0 / GS
    ind2 = np.zeros((G, C), np.float32)
    for c in range(C):
        ind2[c // GS, c] = 1.0
    ind1_d = nc.inline_tensor(ind1, name="ind1")
    ind2_d = nc.inline_tensor(ind2, name="ind2")
    ident_d = nc.inline_tensor(
        np.eye(128).astype(ml_dtypes.bfloat16), name="ident"
    )
    ones8_d = nc.dram_tensor("ones8", [128, 2, 512], FP8, kind="ExternalInput")
    onesr_d = nc.dram_tensor("onesr", [1, 128], BF16, kind="ExternalInput")

    with tile.TileContext(nc) as tc:
        with (
            nc.allow_low_precision(reason="fp8 matmul pipeline, validated 5e-3"),
            tc.tile_pool(name="persist", bufs=1) as pers,
            tc.tile_pool(name="small", bufs=4) as spool,
            tc.tile_pool(name="ps", bufs=6, space="PSUM") as psp,
            tc.tile_pool(name="psden", bufs=2, space="PSUM") as psd,
        ):
            # ================= input DMA schedule =========================
            # sync queue:   x slot0 a/b, slot2 a/b, qa, pw
            # scalar queue: ones8, x slot1 a/b, slot3 a/b, qv
            # gpsimd queue: ind1, ind2, pb, onesr (small, early)
            x_sb = pers.tile([128, CT, NPIX], F32)
            ones8_sb = pers.tile([128, 2, 512], FP8)
            qa_sb = pers.tile([128, CT, C], FP8)
            qv_sb = pers.tile([128, CT, C], FP8)
            pw_sb = pers.tile([128, CT, C], FP8)
            pb_sb = pers.tile([128, CT], F32)
            ind1_sb = pers.tile([128, CT * G], F32)
            ind2_sb = pers.tile([G, C], F32)
            ones_row = pers.tile([1, 128], BF16)

            def xdma(eng, s, h):
                return eng.dma_start(
                    x_sb[:, s, h * 512 : (h + 1) * 512],
                    x_d[:, s, h * 512 : (h + 1) * 512],
                )

            # first-half x slices lead both queues (stats pace with arrival);
            # qa rides between them (needed when the first t matmuls fire);
            # ones8 first on scalar (warm-up gate); pw/qv data-serialized
            # behind the last x slice of their queue.
            xdma(nc.sync, 0, 0)
            xdma(nc.sync, 2, 0)
            nc.sync.dma_start(qa_sb[:], qa_d[:])
            xdma(nc.sync, 0, 1)
            last_sync = xdma(nc.sync, 2, 1)
            d = nc.sync.dma_start(pw_sb[:], pw_d[:])
            add_dep_helper(d.ins, last_sync.ins, sync=True,
                           reason="let x DMA finish first")

            nc.scalar.dma_start(ones8_sb[:], ones8_d[:])
            xdma(nc.scalar, 1, 0)
            xdma(nc.scalar, 3, 0)
            xdma(nc.scalar, 1, 1)
            last_scalar = xdma(nc.scalar, 3, 1)
            d = nc.scalar.dma_start(qv_sb[:], qv_d[:])
            add_dep_helper(d.ins, last_scalar.ins, sync=True,
                           reason="let x DMA finish first")

            ident_sb = pers.tile([128, 128], BF16)
            nc.gpsimd.dma_start(ind1_sb[:], ind1_d[:])
            nc.gpsimd.dma_start(ind2_sb[:], ind2_d[:])
            nc.gpsimd.dma_start(pb_sb[:], pb_d[:])
            nc.gpsimd.dma_start(ones_row[:], onesr_d[:])
            nc.gpsimd.dma_start(ident_sb[:], ident_d[:])

            # ================= small constants ============================
            eps_sb = pers.tile([G, 1], F32)
            nc.vector.memset(eps_sb[:], EPS)
            eshift_sb = pers.tile([128, 1], F32)
            nc.vector.memset(eshift_sb[:], ESHIFT)
            zero128 = pers.tile([128, 1], F32)
            nc.vector.memset(zero128[:], 0.0)
            # grow holds per-ct [rstd, mean*rstd] columns; zero so the per-ct
            # bc matmuls never multiply ind2 zeros against uninitialized SBUF.
            grow = pers.tile([G, CT * 2], F32)
            nc.vector.memset(grow[:], 0.0)

            # Dummy sqrt: pulls the sqrt-table load into the DMA window; the
            # exp table is loaded by a second dummy after the last xn slice.
            dummy_sb = pers.tile([G, 1], F32)
            nc.scalar.activation(dummy_sb[:], eps_sb[:], AF.Sqrt)

            # ================= PE warm-up =================================
            warm_ps = psp.tile([128, 512], F32, tag="ps")

            def warm(n, after=None):
                for _ in range(n):
                    mm = nc.tensor.matmul(
                        warm_ps[:],
                        ones8_sb[:, 0:2, 0:128],
                        ones8_sb[:, 0:2, :],
                        start=True,
                        stop=True,
                        perf_mode=DR,
                    )
                    if after is not None:
                        add_dep_helper(mm.ins, after.ins, sync=True,
                                       reason="clock-keeping filler ordering")

            warm(WARM_MMS)

            # ================= group norm, per-ct pipelined ===============
            # mv[:, ct, :] = [mean, E[x^2]] per channel.
            mv = pers.tile([128, CT, 2], F32)
            st6 = pers.tile([128, CT, 2, 6], F32)

            # all stats on DVE, in expected arrival order (slots 0/2 on the
            # sync queue, 1/3 on the scalar queue, interleaved).
            def stat(s, h):
                nc.vector.bn_stats(
                    st6[:, s, h, :], x_sb[:, s, h * 512 : (h + 1) * 512]
                )

            aggr_done = {}

            def aggr(s):
                aggr_done[s] = nc.vector.bn_aggr(mv[:, s, :], st6[:, s, :, :])

            stat(0, 0)
            stat(1, 0)
            stat(2, 0)
            stat(3, 0)
            stat(0, 1)
            aggr(0)
            stat(1, 1)
            aggr(1)
            stat(2, 1)
            aggr(2)
            stat(3, 1)
            aggr(3)

            # per-ct chains: gpsimd computes statcols/var/rstd/mean*rstd,
            # PE does the two tiny reduction/broadcast matmuls, DVE/gpsimd
            # copy psum slices out.
            gsum_ps = psd.tile([G, CT * 2], F32, tag="psd")
            bcall = psd.tile([128, CT, 2], F32, tag="psd")
            gs_sb = pers.tile([G, CT * 2], F32)
            sca = pers.tile([128, CT, 3], F32)

            def chain_ct(s):
                # bn_aggr gives [mean, var]; statcol1 = mean^2 + var.
                # The tiny chain ops run on the otherwise-idle GpSimd so the
                # DVE keeps pace with arriving bn_stats slices.
                sq = spool.tile([128, 1], F32, tag=f"sq{s}")
                nc.gpsimd.tensor_mul(sq[:], mv[:, s, 0:1], mv[:, s, 0:1])
                nc.gpsimd.tensor_add(mv[:, s, 1:2], mv[:, s, 1:2], sq[:])
                # group reduce: gsum_ps[:, 2s:2s+2] = [mean_g, E_g[x^2]]
                nc.tensor.matmul(
                    gsum_ps[:, 2 * s : 2 * s + 2],
                    ind1_sb[:, s * G : (s + 1) * G],
                    mv[:, s, :],
                    start=True,
                    stop=True,
                )
                gsl = gs_sb[:, 2 * s : 2 * s + 2]
                nc.scalar.activation(gsl, gsum_ps[:, 2 * s : 2 * s + 2],
                                     AF.Identity)
                var = spool.tile([G, 1], F32, tag=f"var{s}")
                nc.gpsimd.tensor_mul(
                    var[:], gs_sb[:, 2 * s : 2 * s + 1],
                    gs_sb[:, 2 * s : 2 * s + 1],
                )
                nc.gpsimd.tensor_sub(
                    var[:], var[:], gs_sb[:, 2 * s + 1 : 2 * s + 2]
                )  # mean^2 - E[x^2] = -var
                # rstd = 1/sqrt(var+eps): Sqrt on Act (table preloaded by the
                # dummy), reciprocal on DVE.
                rsl = grow[:, 2 * s : 2 * s + 1]
                nc.scalar.activation(
                    rsl, var[:], AF.Sqrt, scale=-1.0, bias=eps_sb[:]
                )
                nc.vector.reciprocal(rsl, rsl)
                # grow col1 = mean * rstd (xn subtracts it / Act adds neg)
                nc.gpsimd.tensor_mul(
                    grow[:, 2 * s + 1 : 2 * s + 2],
                    gs_sb[:, 2 * s : 2 * s + 1],
                    rsl,
                )
                # broadcast group rows to channels
                nc.tensor.matmul(
                    bcall[:, s, :],
                    ind2_sb[:, s * 128 : (s + 1) * 128],
                    grow[:, 2 * s : 2 * s + 2],
                    start=True,
                    stop=True,
                )
                nc.scalar.activation(sca[:, s, 0:2], bcall[:, s, :], AF.Identity)
                if s in XN_ACT:
                    nc.gpsimd.tensor_sub(
                        sca[:, s, 2:3], zero128[:], sca[:, s, 1:2]
                    )

            chain_ct(1)
            chain_ct(0)
            chain_ct(3)
            chain_ct(2)

            # xn = x * rstd - mean*rstd, cast to fp8
            xn_sb = pers.tile([128, CT, NPIX], FP8)

            def xn_op(s, h):
                dst = xn_sb[:, s, h * 512 : (h + 1) * 512]
                src = x_sb[:, s, h * 512 : (h + 1) * 512]
                if s in XN_ACT:
                    return nc.scalar.activation(
                        dst, src, AF.Identity,
                        scale=sca[:, s, 0:1], bias=sca[:, s, 2:3],
                    )
                return nc.vector.tensor_scalar(
                    out=dst,
                    in0=src,
                    scalar1=sca[:, s, 0:1],
                    scalar2=sca[:, s, 1:2],
                    op0=AOP.mult,
                    op1=AOP.subtract,
                )

            xn_op(1, 0)
            xn_op(0, 0)
            xn_op(3, 0)
            xn_op(1, 1)
            last_act_xn = xn_op(0, 1)
            xn_op(2, 0)
            xn_op(3, 1)
            xn_op(2, 1)

            # Dummy exp chained behind the last Act xn: loads the exp table
            # during the t-GEMM window, before the first softmax Exp.
            dexp = nc.scalar.activation(dummy_sb[:], eps_sb[:], AF.Exp)
            add_dep_helper(dexp.ins, last_act_xn.ins, sync=True,
                           reason="exp table load after sqrt/xn phase")

            warm(FILL_A, after=aggr_done[1])

            # ================= t = (16A) @ xn ============================
            # ci0 (k-cts 0,1) is emitted for all co first so the PE can run
            # it while slot2/3's xn is still in flight; ci1 stops follow.
            t_sb = pers.tile([128, CT, NPIX], FP8)
            t_ps = {}
            for nh in range(NH):
                if nh == 0:
                    fill_anchor = None
                    for co in range(CT):
                        ps = psp.tile([128, 512], F32, tag="ps")
                        t_ps[co] = ps
                        fill_anchor = nc.tensor.matmul(
                            ps[:],
                            qa_sb[:, 0:2, co * 128 : (co + 1) * 128],
                            xn_sb[:, 0:2, 0:512],
                            start=True,
                            stop=False,
                            perf_mode=DR,
                        )
                    warm(FILL_B, after=fill_anchor)
                    for co in range(CT):
                        nc.tensor.matmul(
                            t_ps[co][:],
                            qa_sb[:, 2:4, co * 128 : (co + 1) * 128],
                            xn_sb[:, 2:4, 0:512],
                            start=False,
                            stop=True,
                            perf_mode=DR,
                        )
                        nc.vector.tensor_copy(t_sb[:, co, 0:512], t_ps[co][:])
                else:
                    for co in range(CT):
                        ps = psp.tile([128, 512], F32, tag="ps")
                        for i in range(2):
                            nc.tensor.matmul(
                                ps[:],
                                qa_sb[:, 2 * i : 2 * i + 2,
                                      co * 128 : (co + 1) * 128],
                                xn_sb[:, 2 * i : 2 * i + 2, 512:1024],
                                start=(i == 0),
                                stop=(i == 1),
                                perf_mode=DR,
                            )
                        nc.vector.tensor_copy(t_sb[:, co, 512:1024], ps[:])

            # ================= v^T = xn^T @ (16 W_v^T) ===================
            vt_sb = pers.tile([128, JT, C], FP8)
            for jt in range(JT):
                ps = psp.tile([128, 512], F32, tag="ps")
                for i in range(2):
                    nc.tensor.matmul(
                        ps[:],
                        xn_sb[:, 2 * i : 2 * i + 2, jt * 128 : (jt + 1) * 128],
                        qv_sb[:, 2 * i : 2 * i + 2, :],
                        start=(i == 0),
                        stop=(i == 1),
                        perf_mode=DR,
                    )
                last_v_evac = nc.vector.tensor_copy(vt_sb[:, jt, :], ps[:])

            # ================= attention =================================
            e_sb = pers.tile([128, JT, NPIX], FP8)
            recip_sb = pers.tile([1, NPIX], F32)
            recip_bf = pers.tile([1, NPIX], BF16)
            rb_sb = pers.tile([128, NPIX], BF16)
            att_sb = pers.tile([128, CT, NPIX], FP8)
            att_ps = {}

            # x residual for pixel-half 1 staged to bf16: rides into the proj
            # PSUM as an identity matmul so the very last evacuations run on
            # the idle Act engine instead of serializing on the DVE.
            xbf_sb = pers.tile([128, CT, 512], BF16)
            for ct in range(CT):
                cp = nc.gpsimd.dma_start(xbf_sb[:, ct, :],
                                         x_sb[:, ct, 512:1024])
                add_dep_helper(cp.ins, last_v_evac.ins, sync=True,
                               reason="keep x->bf16 casts out of the GN window")

            def s_block(nh):
                for jt in range(JT):
                    ps = psp.tile([128, 512], F32, tag="ps")
                    for i in range(2):
                        nc.tensor.matmul(
                            ps[:],
                            xn_sb[:, 2 * i : 2 * i + 2,
                                  jt * 128 : (jt + 1) * 128],
                            t_sb[:, 2 * i : 2 * i + 2,
                                 nh * 512 : (nh + 1) * 512],
                            start=(i == 0),
                            stop=(i == 1),
                            perf_mode=DR,
                        )
                    nc.scalar.activation(
                        e_sb[:, jt, nh * 512 : (nh + 1) * 512],
                        ps[:],
                        AF.Exp,
                        scale=SCALE / WSCALE,
                        bias=eshift_sb[:],
                    )
                dps = psd.tile([1, 512], F32, name=f"den{nh}", tag="psd")
                for jp in range(4):
                    nc.tensor.matmul(
                        dps[:],
                        ones8_sb[:, 0:2, 0:1],
                        e_sb[:, 2 * jp : 2 * jp + 2,
                             nh * 512 : (nh + 1) * 512],
                        start=(jp == 0),
                        stop=(jp == 3),
                        perf_mode=DR,
                    )
                rsl = recip_sb[0:1, nh * 512 : (nh + 1) * 512]
                rscr = spool.tile([1, 512], F32, tag="rscr")
                nc.vector.reciprocal_approx_accurate(rsl, dps[:], rscr[:])
                nc.vector.tensor_copy(
                    recip_bf[0:1, nh * 512 : (nh + 1) * 512], rsl
                )

            def bp_block(nh):
                bp = psd.tile([128, 512], F32, name=f"bp{nh}", tag="psd")
                nc.tensor.matmul(
                    bp[:],
                    ones_row[0:1, :],
                    recip_bf[0:1, nh * 512 : (nh + 1) * 512],
                    start=True,
                    stop=True,
                )
                nc.scalar.activation(
                    rb_sb[:, nh * 512 : (nh + 1) * 512], bp[:], AF.Identity
                )

            def att_mms(nh):
                for ct in range(CT):
                    ps = psp.tile([128, 512], F32, tag="ps")
                    att_ps[(nh, ct)] = ps
                    for jp in range(4):
                        nc.tensor.matmul(
                            ps[:],
                            vt_sb[:, 2 * jp : 2 * jp + 2,
                                  ct * 128 : (ct + 1) * 128],
                            e_sb[:, 2 * jp : 2 * jp + 2,
                                 nh * 512 : (nh + 1) * 512],
                            start=(jp == 0),
                            stop=(jp == 3),
                            perf_mode=DR,
                        )

            def att_evacs(nh):
                for ct in range(CT):
                    nc.vector.tensor_mul(
                        att_sb[:, ct, nh * 512 : (nh + 1) * 512],
                        att_ps[(nh, ct)][:],
                        rb_sb[:, nh * 512 : (nh + 1) * 512],
                    )

            def proj_block(nh):
                # half 0: DVE scalar_tensor_tensor adds proj bias + the f32 x
                # residual (overlaps the half-1 attention matmuls). half 1:
                # the residual rides into PSUM as a bf16 identity matmul and
                # the evacuation is Identity+bias on the idle Act engine, so
                # the endgame doesn't serialize on the DVE.
                for co in range(CT):
                    ps = psp.tile([128, 512], F32, tag="ps")
                    for i in range(2):
                        nc.tensor.matmul(
                            ps[:],
                            pw_sb[:, 2 * i : 2 * i + 2,
                                  co * 128 : (co + 1) * 128],
                            att_sb[:, 2 * i : 2 * i + 2,
                                   nh * 512 : (nh + 1) * 512],
                            start=(i == 0),
                            stop=(i == 1 and nh == 0),
                            perf_mode=DR,
                        )
                    sl = (slice(None), co, slice(nh * 512, (nh + 1) * 512))
                    if nh == 0:
                        nc.vector.scalar_tensor_tensor(
                            out=x_sb[sl],
                            in0=ps[:],
                            scalar=pb_sb[:, co : co + 1],
                            in1=x_sb[sl],
                            op0=AOP.add,
                            op1=AOP.add,
                        )
                    else:
                        nc.tensor.matmul(
                            ps[:], ident_sb[:], xbf_sb[:, co, :],
                            start=False, stop=True,
                        )
                        nc.scalar.activation(
                            x_sb[sl], ps[:], AF.Identity,
                            bias=pb_sb[:, co : co + 1],
                        )
                    eng = nc.sync if co % 2 == 0 else nc.scalar
                    eng.dma_start(y_d[sl], x_sb[sl])

            s_block(0)
            bp_block(0)
            s_block(1)
            att_mms(0)
            att_evacs(0)
            proj_block(0)
            bp_block(1)
            att_mms(1)
            att_evacs(1)
            proj_block(1)

    nc.compile()
    return nc


def _build_f32r():
    """Legacy float32r build, used when the q-bias is nonzero (q/k fold
    invalid) or GroupNorm is non-trivial. Explicit q, k with their biases."""
    nc = bacc.Bacc("TRN2")

    x_d = nc.dram_tensor("x", [128, CT, NPIX], F32, kind="ExternalInput")
    qw_d = nc.dram_tensor("qw", [128, CT, 3 * C], F32R, kind="ExternalInput")
    pw_d = nc.dram_tensor("pw", [128, CT, C], F32R, kind="ExternalInput")
    gnw_d = nc.dram_tensor("gnw", [128, CT], F32, kind="ExternalInput")
    gnb_d = nc.dram_tensor("gnb", [128, CT], F32, kind="ExternalInput")
    qb_d = nc.dram_tensor("qb", [128, CT], F32, kind="ExternalInput")
    kb_d = nc.dram_tensor("kb", [128, CT], F32, kind="ExternalInput")
    pb_d = nc.dram_tensor("pb", [128, CT], F32, kind="ExternalInput")
    y_d = nc.dram_tensor("y", [128, CT, NPIX], F32, kind="ExternalOutput")

    ind1 = np.zeros((128, CT * G), np.float32)
    for ct in range(CT):
        for p in range(128):
            ind1[p, ct * G + ct * 8 + p // GS] = 1.0
    ind2 = np.zeros((G, C), np.float32)
    for c in range(C):
        ind2[c // GS, c] = 1.0
    ind1_d = nc.inline_tensor(ind1, name="ind1")
    ind2_d = nc.inline_tensor(ind2, name="ind2")
    onesc_d = nc.dram_tensor("onesc", [128, 512], F32R, kind="ExternalInput")
    onesr_d = nc.dram_tensor("onesr", [1, 128], F32R, kind="ExternalInput")

    with tile.TileContext(nc) as tc:
        with (
            nc.allow_low_precision(reason="float32r matmul operands"),
            tc.tile_pool(name="persist", bufs=1) as pers,
            tc.tile_pool(name="small", bufs=4) as spool,
            tc.tile_pool(name="ps", bufs=8, space="PSUM") as psp,
        ):
            onesc_sb = pers.tile([128, 512], F32R)
            nc.sync.dma_start(onesc_sb[:], onesc_d[:])

            x_sb = pers.tile([128, CT, NPIX], F32)
            x_dmas = []
            for ct in range(CT):
                for nh in range(NH):
                    x_dmas.append(
                        nc.sync.dma_start(
                            x_sb[:, ct, nh * 512 : (nh + 1) * 512],
                            x_d[:, ct, nh * 512 : (nh + 1) * 512],
                        )
                    )

            gnw_sb = pers.tile([128, CT], F32)
            nc.sync.dma_start(gnw_sb[:], gnw_d[:])
            gnb_sb = pers.tile([128, CT], F32)
            nc.sync.dma_start(gnb_sb[:], gnb_d[:])
            ind1_sb = pers.tile([128, CT * G], F32)
            nc.sync.dma_start(ind1_sb[:], ind1_d[:])
            ind2_sb = pers.tile([G, C], F32)
            nc.sync.dma_start(ind2_sb[:], ind2_d[:])
            qb_sb = pers.tile([128, CT], F32)
            nc.sync.dma_start(qb_sb[:], qb_d[:])
            kb_sb = pers.tile([128, CT], F32)
            nc.sync.dma_start(kb_sb[:], kb_d[:])
            pb_sb = pers.tile([128, CT], F32)
            nc.sync.dma_start(pb_sb[:], pb_d[:])
            ones_row = pers.tile([1, 128], F32R)
            nc.sync.dma_start(ones_row[:], onesr_d[:])

            qw_sb = pers.tile([128, CT, 3 * C], F32R)
            for ci in range(CT):
                d = nc.sync.dma_start(qw_sb[:, ci, :], qw_d[:, ci, :])
                add_dep_helper(d.ins, x_dmas[-1].ins, sync=True,
                               reason="let x DMA finish first")
            pw_sb = pers.tile([128, CT, C], F32R)
            d = nc.sync.dma_start(pw_sb[:], pw_d[:])
            add_dep_helper(d.ins, x_dmas[-1].ins, sync=True,
                           reason="let x DMA finish first")

            eps_sb = pers.tile([G, 1], F32)
            nc.vector.memset(eps_sb[:], EPS)
            ones_row32 = pers.tile([1, 128], F32)
            nc.vector.memset(ones_row32[:], 1.0)

            warm_ps = psp.tile([128, 512], F32, tag="ps")
            for _ in range(38):
                nc.tensor.matmul(
                    warm_ps[:], onesc_sb[:, 0:128], onesc_sb[:], start=True, stop=True
                )

            statcols = pers.tile([128, CT, 2], F32)
            for ct in range(CT):
                st6 = spool.tile([128, 2, 6], F32, tag="st6")
                nc.vector.bn_stats(st6[:, 0, :], x_sb[:, ct, 0:512])
                nc.vector.bn_stats(st6[:, 1, :], x_sb[:, ct, 512:1024])
                mv = spool.tile([128, 2], F32, tag="mv")
                nc.vector.bn_aggr(mv[:], st6[:])
                nc.vector.tensor_copy(statcols[:, ct, 0:1], mv[:, 0:1])
                nc.vector.tensor_mul(statcols[:, ct, 1:2], mv[:, 0:1], mv[:, 0:1])
                nc.vector.tensor_add(
                    statcols[:, ct, 1:2], statcols[:, ct, 1:2], mv[:, 1:2]
                )

            gsum_ps = psp.tile([G, 2], F32, tag="ps")
            for ct in range(CT):
                nc.tensor.matmul(
                    gsum_ps[:],
                    ind1_sb[:, ct * G : (ct + 1) * G],
                    statcols[:, ct, :],
                    start=(ct == 0),
                    stop=(ct == CT - 1),
                )
            gs_sb = spool.tile([G, 2], F32, tag="gs")
            nc.vector.tensor_scalar_mul(gs_sb[:], gsum_ps[:], 1.0 / GS)
            var32 = spool.tile([G, 1], F32, tag="var32")
            nc.vector.tensor_mul(var32[:], gs_sb[:, 0:1], gs_sb[:, 0:1])
            nc.vector.tensor_sub(var32[:], gs_sb[:, 1:2], var32[:])
            grow = pers.tile([G, 2], F32)
            lnv = spool.tile([G, 1], F32, tag="lnv")
            nc.scalar.activation(
                lnv[:], var32[:], mybir.ActivationFunctionType.Ln, bias=eps_sb[:]
            )
            nc.scalar.activation(
                grow[:, 0:1], lnv[:], mybir.ActivationFunctionType.Exp, scale=-0.5
            )
            nc.vector.tensor_mul(grow[:, 1:2], gs_sb[:, 0:1], grow[:, 0:1])

            xn_sb = pers.tile([128, CT, NPIX], F32R)
            chsb = pers.tile([128, CT, 2], F32)
            for ct in range(CT):
                bc_ps = psp.tile([128, 2], F32, tag="ps")
                nc.tensor.matmul(
                    bc_ps[:],
                    ind2_sb[:, ct * 128 : (ct + 1) * 128],
                    grow[:],
                    start=True,
                    stop=True,
                )
                nc.vector.tensor_mul(
                    chsb[:, ct, 0:1], gnw_sb[:, ct : ct + 1], bc_ps[:, 0:1]
                )
                nc.vector.tensor_mul(
                    chsb[:, ct, 1:2], gnw_sb[:, ct : ct + 1], bc_ps[:, 1:2]
                )
                nc.vector.tensor_sub(
                    chsb[:, ct, 1:2], gnb_sb[:, ct : ct + 1], chsb[:, ct, 1:2]
                )
                nc.vector.tensor_scalar(
                    out=xn_sb[:, ct, :],
                    in0=x_sb[:, ct, :],
                    scalar1=chsb[:, ct, 0:1],
                    scalar2=chsb[:, ct, 1:2],
                    op0=mybir.AluOpType.mult,
                    op1=mybir.AluOpType.add,
                )

            q_sb = pers.tile([128, CT, NPIX], F32R)
            k_sb = pers.tile([128, CT, NPIX], F32R)
            for dst, wofs, b_sb in ((q_sb, 0, qb_sb), (k_sb, C, kb_sb)):
                for co in range(CT):
                    for nh in range(NH):
                        ps = psp.tile([128, 512], F32, tag="ps")
                        for ci in range(CT):
                            nc.tensor.matmul(
                                ps[:],
                                qw_sb[:, ci, wofs + co * 128 : wofs + (co + 1) * 128],
                                xn_sb[:, ci, nh * 512 : (nh + 1) * 512],
                                start=(ci == 0),
                                stop=(ci == CT - 1),
                            )
                        nc.scalar.activation(
                            dst[:, co, nh * 512 : (nh + 1) * 512],
                            ps[:],
                            mybir.ActivationFunctionType.Identity,
                            bias=b_sb[:, co : co + 1],
                        )

            vt_sb = pers.tile([128, JT, C], F32R)
            for jt in range(JT):
                ps = psp.tile([128, 512], F32, tag="ps")
                for ci in range(CT):
                    nc.tensor.matmul(
                        ps[:],
                        xn_sb[:, ci, jt * 128 : (jt + 1) * 128],
                        qw_sb[:, ci, 2 * C : 3 * C],
                        start=(ci == 0),
                        stop=(ci == CT - 1),
                    )
                nc.vector.tensor_copy(vt_sb[:, jt, :], ps[:])

            e_sb = pers.tile([128, JT, NPIX], F32R)
            recip_sb = pers.tile([1, NPIX], F32)
            for nh in range(NH):
                dps = psp.tile([1, 512], F32, name=f"den{nh}", tag="ps")
                for jt in range(JT):
                    ps = psp.tile([128, 512], F32, tag="ps")
                    for ci in range(CT):
                        nc.tensor.matmul(
                            ps[:],
                            k_sb[:, ci, jt * 128 : (jt + 1) * 128],
                            q_sb[:, ci, nh * 512 : (nh + 1) * 512],
                            start=(ci == 0),
                            stop=(ci == CT - 1),
                        )
                    esl = e_sb[:, jt, nh * 512 : (nh + 1) * 512]
                    nc.scalar.activation(
                        esl, ps[:], mybir.ActivationFunctionType.Exp, scale=SCALE
                    )
                    nc.tensor.matmul(
                        dps[:],
                        onesc_sb[:, 0:1],
                        esl,
                        start=(jt == 0),
                        stop=(jt == JT - 1),
                    )
                rsl = recip_sb[0:1, nh * 512 : (nh + 1) * 512]
                rscr = spool.tile([1, 512], F32, tag="rscr")
                nc.vector.reciprocal_approx_accurate(rsl, dps[:], rscr[:])

            rb_sb = pers.tile([128, NPIX], F32)
            att_sb = pers.tile([128, CT, NPIX], F32R)
            for nh in range(NH):
                bp = psp.tile([128, 512], F32, name=f"bp{nh}", tag="ps")
                nc.tensor.matmul(
                    bp[:],
                    ones_row32[0:1, :],
                    recip_sb[0:1, nh * 512 : (nh + 1) * 512],
                    start=True,
                    stop=True,
                )
                nc.scalar.activation(
                    rb_sb[:, nh * 512 : (nh + 1) * 512],
                    bp[:],
                    mybir.ActivationFunctionType.Identity,
                )
                for ct in range(CT):
                    ps = psp.tile([128, 512], F32, tag="ps")
                    for jt in range(JT):
                        nc.tensor.matmul(
                            ps[:],
                            vt_sb[:, jt, ct * 128 : (ct + 1) * 128],
                            e_sb[:, jt, nh * 512 : (nh + 1) * 512],
                            start=(jt == 0),
                            stop=(jt == JT - 1),
                        )
                    nc.vector.tensor_mul(
                        att_sb[:, ct, nh * 512 : (nh + 1) * 512],
                        ps[:],
                        rb_sb[:, nh * 512 : (nh + 1) * 512],
                    )

            for nh in range(NH):
                for co in range(CT):
                    ps = psp.tile([128, 512], F32, tag="ps")
                    for ci in range(CT):
                        nc.tensor.matmul(
                            ps[:],
                            pw_sb[:, ci, co * 128 : (co + 1) * 128],
                            att_sb[:, ci, nh * 512 : (nh + 1) * 512],
                            start=(ci == 0),
                            stop=(ci == CT - 1),
                        )
                    sl = (slice(None), co, slice(nh * 512, (nh + 1) * 512))
                    nc.vector.scalar_tensor_tensor(
                        out=x_sb[sl],
                        in0=ps[:],
                        scalar=pb_sb[:, co : co + 1],
                        in1=x_sb[sl],
                        op0=mybir.AluOpType.add,
                        op1=mybir.AluOpType.add,
                    )
                    nc.sync.dma_start(y_d[sl], x_sb[sl])

    nc.compile()
    return nc


def kernel(x, gn_weight, gn_bias, qkv_w, qkv_b, proj_w, proj_b):
    global LAST_RESULTS
    b, c, h, w = x.shape
    assert (b, c, h * w) == (8, C, NPIX)

    qkv_b = np.asarray(qkv_b, np.float32)
    qkv_w = np.asarray(qkv_w, np.float32)
    proj_w = np.asarray(proj_w, np.float32)
    # The per-query bias term cancels in softmax; a nonzero q-bias would
    # contribute a per-key term, so only then fall back to explicit q/k.
    fold_qk = not np.any(qkv_b[0:C])
    # fp8 build assumes gn weight==1 / bias==0 (xn scale/bias come straight
    # from the group broadcast).
    trivial_gn = (
        not np.any(np.asarray(gn_weight, np.float32) != 1.0)
        and not np.any(np.asarray(gn_bias, np.float32))
    )
    use_fp8 = fold_qk and trivial_gn

    key = ("nc", use_fp8)
    if key not in _cache:
        _cache[key] = _build_fp8() if use_fp8 else _build_f32r()
    nc = _cache[key]

    def col(v):  # [512] vector -> [128, CT] per-partition columns
        return np.ascontiguousarray(np.asarray(v, np.float32).reshape(CT, 128).T)

    def wtile(wT, cols, dt=np.float32):  # [c_in, cols] -> [128, CT, cols]
        return np.ascontiguousarray(
            np.asarray(wT).astype(dt).reshape(CT, 128, cols).transpose(1, 0, 2)
        )

    if use_fp8:
        # A^T = W_q^T W_k in fp64 (so that lhsT-layout gives t = W_k^T W_q xn),
        # scaled x16 to keep e4m3 operands in the normal range.
        At = (qkv_w[0:C].astype(np.float64).T @ qkv_w[C : 2 * C].astype(np.float64))
        shared = {
            "qa": wtile(WSCALE * At, C, ml_dtypes.float8_e4m3fn),
            "qv": wtile(WSCALE * qkv_w[2 * C :].T.astype(np.float64), C,
                        ml_dtypes.float8_e4m3fn),
            "pw": wtile(proj_w.T, C, ml_dtypes.float8_e4m3fn),
            # attention rows sum to 1, so att(v + b_v) = att(v) + b_v; fold the
            # v bias through proj into the proj bias on the host.
            "pb": col(proj_b + proj_w @ qkv_b[2 * C :]),
            "ones8": np.full((128, 2, 512), WSCALE, ml_dtypes.float8_e4m3fn),
            "onesr": np.ones((1, 128), ml_dtypes.bfloat16),
        }
    else:
        shared = {
            "qw": wtile(qkv_w.T, 3 * C),
            "pw": wtile(proj_w.T, C),
            "gnw": col(gn_weight),
            "gnb": col(gn_bias),
            "pb": col(proj_b + proj_w @ qkv_b[2 * C :]),
            "qb": col(qkv_b[0:C]),
            "kb": col(qkv_b[C : 2 * C]),
            "onesc": np.ones((128, 512), np.float32),
            "onesr": np.ones((1, 128), np.float32),
        }

    xs = np.asarray(x, np.float32).reshape(b, CT, 128, NPIX)
    in_maps = [
        {"x": np.ascontiguousarray(xs[i].transpose(1, 0, 2)), **shared}
        for i in range(b)
    ]

    res = run_bass_kernel_spmd(
        nc, in_maps, core_ids=list(range(8)), trace=TRACE, **TRACE_KW
    )
    LAST_RESULTS = res
    out = np.stack(
        [r["y"].transpose(1, 0, 2).reshape(c, h, w) for r in res.results]
    )
    return out.astype(np.float32)


# revision 35
# speedup vs baseline: 1.0313x; 1.0313x over previous
"""AttentionBlock (GroupNorm + single-head spatial attention + proj + residual)
on 8 trn2 NeuronCores, data-parallel over the batch (1 image per core).

fp8 build v2: same math as v1 (A = W_k^T W_q folded, x16 host scaling,
E = exp(logits - 1.5)), restructured around the measured v1 trace:
  - input DMA: x streams FIRST on both hardware DGE queues (sync + scalar),
    split into 8 half-ct transfers so bn_stats pipelines with arrival;
    weights follow x in queue order, tiny constants ride the gpsimd queue.
  - GroupNorm fully per-ct pipelined: stats for slot0 on Act (Identity/Square
    with accum_out), slots 1-3 on DVE; the per-group chain (var, rstd via
    pow(var+eps, -0.5), -mean*rstd) runs on the otherwise-idle GpSimd, with
    per-ct gsum/broadcast matmuls, so the first qkv GEMM starts ~6us earlier.
  - proj for pixel-half 0 is issued before attention of half 1, so the output
    DMA streams during the endgame instead of bunching after the last matmul.
  - proj evacuation is a DVE scalar_tensor_tensor (psum + pb + x residual),
    dropping v1's 8 residual identity matmuls and the bf16 x copy.
Measured v1: 60741ns; rel err ~5.4e-3 (budget 2e-2).
"""

import sys

sys.path.insert(0, "/opt/trn_rl_repo")

import numpy as np
import ml_dtypes

import concourse.bass as bass
import concourse.tile as tile
from concourse import bacc, mybir
from concourse.bass_utils import run_bass_kernel_spmd
from concourse.tile_rust import add_dep_helper

F32 = mybir.dt.float32
F32R = mybir.dt.float32r
BF16 = mybir.dt.bfloat16
FP8 = mybir.dt.float8e4
DR = mybir.MatmulPerfMode.DoubleRow
AOP = mybir.AluOpType
AF = mybir.ActivationFunctionType

C = 512          # channels
NPIX = 1024      # pixels per image (32*32)
CT = 4           # channel tiles of 128
JT = 8           # pixel tiles of 128
NH = 2           # halves of NPIX for the 512-wide moving dim
G = 32           # groups
GS = 16          # channels per group
EPS = 1e-5
SCALE = C ** -0.5
WSCALE = 16.0    # host-side scale on A and W_v (and the den ones-vector)
ESHIFT = -1.5    # softmax-invariant logit shift keeping exp() in fp8 range
WARM_MMS = 42    # PE warm-up matmuls during the input-DMA window
FILL_A = 10       # fillers bridging gsum/bc to the first t matmuls
FILL_B = 8       # fillers bridging t ci0 -> ci1
TRACE = False          # set True (from test.py) to capture an NTFF profile
TRACE_KW = {}          # extra kwargs for run_bass_kernel_spmd
LAST_RESULTS = None    # BassKernelResults of the most recent run

_cache = {}

# xn slices computed on Act (need the negated bias column); rest on DVE
XN_ACT = (0, 1)


def _build_fp8():
    nc = bacc.Bacc("TRN2")

    x_d = nc.dram_tensor("x", [128, CT, NPIX], F32, kind="ExternalInput")
    qa_d = nc.dram_tensor("qa", [128, CT, C], FP8, kind="ExternalInput")
    qv_d = nc.dram_tensor("qv", [128, CT, C], FP8, kind="ExternalInput")
    pw_d = nc.dram_tensor("pw", [128, CT, C], FP8, kind="ExternalInput")
    pb_d = nc.dram_tensor("pb", [128, CT], F32, kind="ExternalInput")
    y_d = nc.dram_tensor("y", [128, CT, NPIX], F32, kind="ExternalOutput")

    # Indicator constants for the cross-partition group reductions.
    # ind1 carries the 1/GS mean factor so the gsum psum is already [mean,
    # E[x^2]] per group; ind2 broadcasts group rows back to channels.
    ind1 = np.zeros((128, CT * G), np.float32)
    for ct in range(CT):
        for p in range(128):
            ind1[p, ct * G + ct * 8 + p // GS] = 1.0 / GS
    ind2 = np.zeros((G, C), np.float32)
    for c in range(C):
        ind2[c // GS, c] = 1.0
    ind1_d = nc.inline_tensor(ind1, name="ind1")
    ind2_d = nc.inline_tensor(ind2, name="ind2")
    ident_d = nc.inline_tensor(
        np.eye(128).astype(ml_dtypes.bfloat16), name="ident"
    )
    ones8_d = nc.dram_tensor("ones8", [128, 2, 128], FP8, kind="ExternalInput")
    onesr_d = nc.dram_tensor("onesr", [1, 128], BF16, kind="ExternalInput")

    with tile.TileContext(nc) as tc:
        with (
            nc.allow_low_precision(reason="fp8 matmul pipeline, validated 5e-3"),
            tc.tile_pool(name="persist", bufs=1) as pers,
            tc.tile_pool(name="small", bufs=4) as spool,
            tc.tile_pool(name="ps", bufs=6, space="PSUM") as psp,
            tc.tile_pool(name="psden", bufs=2, space="PSUM") as psd,
        ):
            # ================= input DMA schedule =========================
            # sync queue:   x slot0 a/b, slot2 a/b, qa, pw
            # scalar queue: ones8, x slot1 a/b, slot3 a/b, qv
            # gpsimd queue: ind1, ind2, pb, onesr (small, early)
            x_sb = pers.tile([128, CT, NPIX], F32)
            ones8_sb = pers.tile([128, 2, 128], FP8)
            qa_sb = pers.tile([128, CT, C], FP8)
            qv_sb = pers.tile([128, CT, C], FP8)
            pw_sb = pers.tile([128, CT, C], FP8)
            pb_sb = pers.tile([128, CT], F32)
            ind1_sb = pers.tile([128, CT * G], F32)
            ind2_sb = pers.tile([G, C], F32)
            ones_row = pers.tile([1, 128], BF16)

            def xdma(eng, s, h):
                return eng.dma_start(
                    x_sb[:, s, h * 512 : (h + 1) * 512],
                    x_d[:, s, h * 512 : (h + 1) * 512],
                )

            # first-half x slices lead both queues (stats pace with arrival);
            # qa rides between them (needed when the first t matmuls fire);
            # ones8 first on scalar (warm-up gate); pw/qv data-serialized
            # behind the last x slice of their queue.
            xdma(nc.sync, 0, 0)
            xdma(nc.sync, 2, 0)
            nc.sync.dma_start(qa_sb[:], qa_d[:])
            xdma(nc.sync, 0, 1)
            last_sync = xdma(nc.sync, 2, 1)
            d = nc.sync.dma_start(pw_sb[:], pw_d[:])
            add_dep_helper(d.ins, last_sync.ins, sync=True,
                           reason="let x DMA finish first")

            nc.scalar.dma_start(ones8_sb[:], ones8_d[:])
            xdma(nc.scalar, 1, 0)
            xdma(nc.scalar, 3, 0)
            xdma(nc.scalar, 1, 1)
            last_scalar = xdma(nc.scalar, 3, 1)
            d = nc.scalar.dma_start(qv_sb[:], qv_d[:])
            add_dep_helper(d.ins, last_scalar.ins, sync=True,
                           reason="let x DMA finish first")

            ident_sb = pers.tile([128, 128], BF16)
            nc.gpsimd.dma_start(ind1_sb[:], ind1_d[:])
            nc.gpsimd.dma_start(ind2_sb[:], ind2_d[:])
            nc.gpsimd.dma_start(pb_sb[:], pb_d[:])
            nc.gpsimd.dma_start(ones_row[:], onesr_d[:])
            nc.gpsimd.dma_start(ident_sb[:], ident_d[:])

            # ================= small constants ============================
            eps_sb = pers.tile([G, 1], F32)
            nc.vector.memset(eps_sb[:], EPS)
            eshift_sb = pers.tile([128, 1], F32)
            nc.vector.memset(eshift_sb[:], ESHIFT)
            zero128 = pers.tile([128, 1], F32)
            nc.vector.memset(zero128[:], 0.0)
            # grow holds per-ct [rstd, mean*rstd] columns; zero so the per-ct
            # bc matmuls never multiply ind2 zeros against uninitialized SBUF.
            grow = pers.tile([G, CT * 2], F32)
            nc.vector.memset(grow[:], 0.0)

            # Dummy sqrt: pulls the sqrt-table load into the DMA window; the
            # exp table is loaded by a second dummy after the last xn slice.
            dummy_sb = pers.tile([G, 1], F32)
            nc.scalar.activation(dummy_sb[:], eps_sb[:], AF.Sqrt)

            # ================= PE warm-up =================================
            warm_ps = psp.tile([128, 512], F32, tag="ps")

            def warm(n, after=None):
                for _ in range(n):
                    mm = nc.tensor.matmul(
                        warm_ps[:, 0:128],
                        ones8_sb[:, 0:2, :],
                        ones8_sb[:, 0:2, :],
                        start=True,
                        stop=True,
                        perf_mode=DR,
                    )
                    if after is not None:
                        add_dep_helper(mm.ins, after.ins, sync=True,
                                       reason="clock-keeping filler ordering")

            warm(WARM_MMS)

            # ================= group norm, per-ct pipelined ===============
            # mv[:, ct, :] = [mean, E[x^2]] per channel.
            mv = pers.tile([128, CT, 2], F32)
            st6 = pers.tile([128, CT, 2, 6], F32)

            # all stats on DVE, in expected arrival order (slots 0/2 on the
            # sync queue, 1/3 on the scalar queue, interleaved).
            def stat(s, h):
                nc.vector.bn_stats(
                    st6[:, s, h, :], x_sb[:, s, h * 512 : (h + 1) * 512]
                )

            aggr_done = {}

            def aggr(s):
                aggr_done[s] = nc.vector.bn_aggr(mv[:, s, :], st6[:, s, :, :])

            stat(0, 0)
            stat(1, 0)
            stat(2, 0)
            stat(3, 0)
            stat(0, 1)
            aggr(0)
            stat(1, 1)
            aggr(1)
            stat(2, 1)
            aggr(2)
            stat(3, 1)
            aggr(3)

            # per-ct chains: gpsimd computes statcols/var/rstd/mean*rstd,
            # PE does the two tiny reduction/broadcast matmuls, DVE/gpsimd
            # copy psum slices out.
            gsum_ps = psd.tile([G, CT * 2], F32, tag="psd")
            bcall = psd.tile([128, CT, 2], F32, tag="psd")
            gs_sb = pers.tile([G, CT * 2], F32)
            sca = pers.tile([128, CT, 3], F32)

            def chain_ct(s):
                # bn_aggr gives [mean, var]; statcol1 = mean^2 + var.
                # The tiny chain ops run on the otherwise-idle GpSimd so the
                # DVE keeps pace with arriving bn_stats slices.
                sq = spool.tile([128, 1], F32, tag=f"sq{s}")
                nc.gpsimd.tensor_mul(sq[:], mv[:, s, 0:1], mv[:, s, 0:1])
                nc.gpsimd.tensor_add(mv[:, s, 1:2], mv[:, s, 1:2], sq[:])
                # group reduce: gsum_ps[:, 2s:2s+2] = [mean_g, E_g[x^2]]
                nc.tensor.matmul(
                    gsum_ps[:, 2 * s : 2 * s + 2],
                    ind1_sb[:, s * G : (s + 1) * G],
                    mv[:, s, :],
                    start=True,
                    stop=True,
                )
                gsl = gs_sb[:, 2 * s : 2 * s + 2]
                nc.scalar.activation(gsl, gsum_ps[:, 2 * s : 2 * s + 2],
                                     AF.Identity)
                var = spool.tile([G, 1], F32, tag=f"var{s}")
                nc.gpsimd.tensor_mul(
                    var[:], gs_sb[:, 2 * s : 2 * s + 1],
                    gs_sb[:, 2 * s : 2 * s + 1],
                )
                nc.gpsimd.tensor_sub(
                    var[:], var[:], gs_sb[:, 2 * s + 1 : 2 * s + 2]
                )  # mean^2 - E[x^2] = -var
                # rstd = 1/sqrt(var+eps): Sqrt on Act (table preloaded by the
                # dummy), reciprocal on DVE.
                rsl = grow[:, 2 * s : 2 * s + 1]
                sqrt_op = nc.scalar.activation(
                    rsl, var[:], AF.Sqrt, scale=-1.0, bias=eps_sb[:]
                )
                nc.vector.reciprocal(rsl, rsl)  # noqa: chain returns sqrt
                # grow col1 = mean * rstd (xn subtracts it / Act adds neg)
                nc.gpsimd.tensor_mul(
                    grow[:, 2 * s + 1 : 2 * s + 2],
                    gs_sb[:, 2 * s : 2 * s + 1],
                    rsl,
                )
                # broadcast group rows to channels
                nc.tensor.matmul(
                    bcall[:, s, :],
                    ind2_sb[:, s * 128 : (s + 1) * 128],
                    grow[:, 2 * s : 2 * s + 2],
                    start=True,
                    stop=True,
                )
                nc.scalar.activation(sca[:, s, 0:2], bcall[:, s, :], AF.Identity)
                if s in XN_ACT:
                    nc.gpsimd.tensor_sub(
                        sca[:, s, 2:3], zero128[:], sca[:, s, 1:2]
                    )
                return sqrt_op

            chain_ct(0)
            chain_ct(1)
            chain_ct(2)
            last_sqrt = chain_ct(3)

            # xn = x * rstd - mean*rstd, cast to fp8
            xn_sb = pers.tile([128, CT, NPIX], FP8)

            def xn_op(s, h):
                dst = xn_sb[:, s, h * 512 : (h + 1) * 512]
                src = x_sb[:, s, h * 512 : (h + 1) * 512]
                if s in XN_ACT and h == 0:
                    return nc.scalar.activation(
                        dst, src, AF.Identity,
                        scale=sca[:, s, 0:1], bias=sca[:, s, 2:3],
                    )
                return nc.vector.tensor_scalar(
                    out=dst,
                    in0=src,
                    scalar1=sca[:, s, 0:1],
                    scalar2=sca[:, s, 1:2],
                    op0=AOP.mult,
                    op1=AOP.subtract,
                )

            xn_op(0, 0)
            xn_op(1, 0)
            xn_op(2, 0)
            xn_op(3, 0)
            xn_op(0, 1)
            xn_op(1, 1)
            xn_op(2, 1)
            xn_op(3, 1)

            # Dummy exp chained behind the last sqrt: loads the exp table
            # during the t-GEMM window without evicting the sqrt table early.
            dexp = nc.scalar.activation(dummy_sb[:], eps_sb[:], AF.Exp)
            add_dep_helper(dexp.ins, last_sqrt.ins, sync=True,
                           reason="exp table load after the sqrt phase")

            warm(FILL_A, after=aggr_done[1])

            # ================= t = (16A) @ xn ============================
            # ci0 (k-cts 0,1) is emitted for all co first so the PE can run
            # it while slot2/3's xn is still in flight; ci1 stops follow.
            t_sb = pers.tile([128, CT, NPIX], FP8)
            t_ps = {}
            for nh in range(NH):
                if nh == 0:
                    fill_anchor = None
                    for co in range(CT):
                        ps = psp.tile([128, 512], F32, tag="ps")
                        t_ps[co] = ps
                        fill_anchor = nc.tensor.matmul(
                            ps[:],
                            qa_sb[:, 0:2, co * 128 : (co + 1) * 128],
                            xn_sb[:, 0:2, 0:512],
                            start=True,
                            stop=False,
                            perf_mode=DR,
                        )
                    warm(FILL_B, after=fill_anchor)
                    for co in range(CT):
                        nc.tensor.matmul(
                            t_ps[co][:],
                            qa_sb[:, 2:4, co * 128 : (co + 1) * 128],
                            xn_sb[:, 2:4, 0:512],
                            start=False,
                            stop=True,
                            perf_mode=DR,
                        )
                        nc.scalar.activation(
                            t_sb[:, co, 0:512], t_ps[co][:], AF.Identity
                        )
                else:
                    for co in range(CT):
                        ps = psp.tile([128, 512], F32, tag="ps")
                        for i in range(2):
                            nc.tensor.matmul(
                                ps[:],
                                qa_sb[:, 2 * i : 2 * i + 2,
                                      co * 128 : (co + 1) * 128],
                                xn_sb[:, 2 * i : 2 * i + 2, 512:1024],
                                start=(i == 0),
                                stop=(i == 1),
                                perf_mode=DR,
                            )
                        nc.scalar.activation(
                            t_sb[:, co, 512:1024], ps[:], AF.Identity
                        )

            # ================= v^T = xn^T @ (16 W_v^T) ===================
            vt_sb = pers.tile([128, JT, C], FP8)
            for jt in range(JT):
                ps = psp.tile([128, 512], F32, tag="ps")
                for i in range(2):
                    nc.tensor.matmul(
                        ps[:],
                        xn_sb[:, 2 * i : 2 * i + 2, jt * 128 : (jt + 1) * 128],
                        qv_sb[:, 2 * i : 2 * i + 2, :],
                        start=(i == 0),
                        stop=(i == 1),
                        perf_mode=DR,
                    )
                last_v_evac = nc.vector.tensor_copy(vt_sb[:, jt, :], ps[:])

            # ================= attention =================================
            e_sb = pers.tile([128, JT, NPIX], FP8)
            recip_sb = pers.tile([1, NPIX], F32)
            recip_bf = pers.tile([1, NPIX], BF16)
            rb_sb = pers.tile([128, NPIX], BF16)
            att_sb = pers.tile([128, CT, NPIX], FP8)
            att_ps = {}

            # x residual for pixel-half 1 staged to bf16: rides into the proj
            # PSUM as an identity matmul so the very last evacuations run on
            # the idle Act engine instead of serializing on the DVE.
            xbf_sb = pers.tile([128, CT, 512], BF16)
            for ct in range(CT):
                cp = nc.gpsimd.dma_start(xbf_sb[:, ct, :],
                                         x_sb[:, ct, 512:1024])
                add_dep_helper(cp.ins, last_v_evac.ins, sync=True,
                               reason="keep x->bf16 casts out of the GN window")

            def s_block(nh):
                for jt in range(JT):
                    ps = psp.tile([128, 512], F32, tag="ps")
                    for i in range(2):
                        nc.tensor.matmul(
                            ps[:],
                            xn_sb[:, 2 * i : 2 * i + 2,
                                  jt * 128 : (jt + 1) * 128],
                            t_sb[:, 2 * i : 2 * i + 2,
                                 nh * 512 : (nh + 1) * 512],
                            start=(i == 0),
                            stop=(i == 1),
                            perf_mode=DR,
                        )
                    nc.scalar.activation(
                        e_sb[:, jt, nh * 512 : (nh + 1) * 512],
                        ps[:],
                        AF.Exp,
                        scale=SCALE / WSCALE,
                        bias=eshift_sb[:],
                    )
                dps = psd.tile([1, 512], F32, name=f"den{nh}", tag="psd")
                for jp in range(4):
                    nc.tensor.matmul(
                        dps[:],
                        ones8_sb[:, 0:2, 0:1],
                        e_sb[:, 2 * jp : 2 * jp + 2,
                             nh * 512 : (nh + 1) * 512],
                        start=(jp == 0),
                        stop=(jp == 3),
                        perf_mode=DR,
                    )
                rsl = recip_sb[0:1, nh * 512 : (nh + 1) * 512]
                rscr = spool.tile([1, 512], F32, tag="rscr")
                nc.vector.reciprocal_approx_accurate(rsl, dps[:], rscr[:])
                nc.vector.tensor_copy(
                    recip_bf[0:1, nh * 512 : (nh + 1) * 512], rsl
                )

            def bp_block(nh):
                bp = psd.tile([128, 512], F32, name=f"bp{nh}", tag="psd")
                bpmm = nc.tensor.matmul(
                    bp[:],
                    ones_row[0:1, :],
                    recip_bf[0:1, nh * 512 : (nh + 1) * 512],
                    start=True,
                    stop=True,
                )
                nc.scalar.activation(
                    rb_sb[:, nh * 512 : (nh + 1) * 512], bp[:], AF.Identity
                )
                return bpmm

            def att_mms(nh, bpmm=None):
                for ct in range(CT):
                    ps = psp.tile([128, 512], F32, tag="ps")
                    att_ps[(nh, ct)] = ps
                    for jp in range(4):
                        mm = nc.tensor.matmul(
                            ps[:],
                            vt_sb[:, 2 * jp : 2 * jp + 2,
                                  ct * 128 : (ct + 1) * 128],
                            e_sb[:, 2 * jp : 2 * jp + 2,
                                 nh * 512 : (nh + 1) * 512],
                            start=(jp == 0),
                            stop=(jp == 3),
                            perf_mode=DR,
                        )
                        if bpmm is not None and ct == 2 and jp == 0:
                            add_dep_helper(
                                mm.ins, bpmm.ins, sync=True,
                                reason="rb broadcast before att evacs need it")

            def att_evacs(nh):
                for ct in range(CT):
                    nc.vector.tensor_mul(
                        att_sb[:, ct, nh * 512 : (nh + 1) * 512],
                        att_ps[(nh, ct)][:],
                        rb_sb[:, nh * 512 : (nh + 1) * 512],
                    )

            def proj_block(nh):
                # half 0: DVE scalar_tensor_tensor adds proj bias + the f32 x
                # residual (overlaps the half-1 attention matmuls). half 1:
                # the residual rides into PSUM as a bf16 identity matmul and
                # the evacuation is Identity+bias on the idle Act engine, so
                # the endgame doesn't serialize on the DVE.
                for co in range(CT):
                    ps = psp.tile([128, 512], F32, tag="ps")
                    for i in range(2):
                        nc.tensor.matmul(
                            ps[:],
                            pw_sb[:, 2 * i : 2 * i + 2,
                                  co * 128 : (co + 1) * 128],
                            att_sb[:, 2 * i : 2 * i + 2,
                                   nh * 512 : (nh + 1) * 512],
                            start=(i == 0),
                            stop=(i == 1 and nh == 0),
                            perf_mode=DR,
                        )
                    sl = (slice(None), co, slice(nh * 512, (nh + 1) * 512))
                    if nh == 0:
                        nc.vector.scalar_tensor_tensor(
                            out=x_sb[sl],
                            in0=ps[:],
                            scalar=pb_sb[:, co : co + 1],
                            in1=x_sb[sl],
                            op0=AOP.add,
                            op1=AOP.add,
                        )
                    else:
                        nc.tensor.matmul(
                            ps[:], ident_sb[:], xbf_sb[:, co, :],
                            start=False, stop=True,
                        )
                        nc.scalar.activation(
                            x_sb[sl], ps[:], AF.Identity,
                            bias=pb_sb[:, co : co + 1],
                        )
                    eng = nc.sync if co % 2 == 0 else nc.scalar
                    eng.dma_start(y_d[sl], x_sb[sl])

            s_block(0)
            bp0 = bp_block(0)
            s_block(1)
            att_mms(0, bp0)
            att_evacs(0)
            proj_block(0)
            bp1 = bp_block(1)
            att_mms(1, bp1)
            att_evacs(1)
            proj_block(1)

    nc.compile()
    return nc


def _build_f32r():
    """Legacy float32r build, used when the q-bias is nonzero (q/k fold
    invalid) or GroupNorm is non-trivial. Explicit q, k with their biases."""
    nc = bacc.Bacc("TRN2")

    x_d = nc.dram_tensor("x", [128, CT, NPIX], F32, kind="ExternalInput")
    qw_d = nc.dram_tensor("qw", [128, CT, 3 * C], F32R, kind="ExternalInput")
    pw_d = nc.dram_tensor("pw", [128, CT, C], F32R, kind="ExternalInput")
    gnw_d = nc.dram_tensor("gnw", [128, CT], F32, kind="ExternalInput")
    gnb_d = nc.dram_tensor("gnb", [128, CT], F32, kind="ExternalInput")
    qb_d = nc.dram_tensor("qb", [128, CT], F32, kind="ExternalInput")
    kb_d = nc.dram_tensor("kb", [128, CT], F32, kind="ExternalInput")
    pb_d = nc.dram_tensor("pb", [128, CT], F32, kind="ExternalInput")
    y_d = nc.dram_tensor("y", [128, CT, NPIX], F32, kind="ExternalOutput")

    ind1 = np.zeros((128, CT * G), np.float32)
    for ct in range(CT):
        for p in range(128):
            ind1[p, ct * G + ct * 8 + p // GS] = 1.0
    ind2 = np.zeros((G, C), np.float32)
    for c in range(C):
        ind2[c // GS, c] = 1.0
    ind1_d = nc.inline_tensor(ind1, name="ind1")
    ind2_d = nc.inline_tensor(ind2, name="ind2")
    onesc_d = nc.dram_tensor("onesc", [128, 512], F32R, kind="ExternalInput")
    onesr_d = nc.dram_tensor("onesr", [1, 128], F32R, kind="ExternalInput")

    with tile.TileContext(nc) as tc:
        with (
            nc.allow_low_precision(reason="float32r matmul operands"),
            tc.tile_pool(name="persist", bufs=1) as pers,
            tc.tile_pool(name="small", bufs=4) as spool,
            tc.tile_pool(name="ps", bufs=8, space="PSUM") as psp,
        ):
            onesc_sb = pers.tile([128, 512], F32R)
            nc.sync.dma_start(onesc_sb[:], onesc_d[:])

            x_sb = pers.tile([128, CT, NPIX], F32)
            x_dmas = []
            for ct in range(CT):
                for nh in range(NH):
                    x_dmas.append(
                        nc.sync.dma_start(
                            x_sb[:, ct, nh * 512 : (nh + 1) * 512],
                            x_d[:, ct, nh * 512 : (nh + 1) * 512],
                        )
                    )

            gnw_sb = pers.tile([128, CT], F32)
            nc.sync.dma_start(gnw_sb[:], gnw_d[:])
            gnb_sb = pers.tile([128, CT], F32)
            nc.sync.dma_start(gnb_sb[:], gnb_d[:])
            ind1_sb = pers.tile([128, CT * G], F32)
            nc.sync.dma_start(ind1_sb[:], ind1_d[:])
            ind2_sb = pers.tile([G, C], F32)
            nc.sync.dma_start(ind2_sb[:], ind2_d[:])
            qb_sb = pers.tile([128, CT], F32)
            nc.sync.dma_start(qb_sb[:], qb_d[:])
            kb_sb = pers.tile([128, CT], F32)
            nc.sync.dma_start(kb_sb[:], kb_d[:])
            pb_sb = pers.tile([128, CT], F32)
            nc.sync.dma_start(pb_sb[:], pb_d[:])
            ones_row = pers.tile([1, 128], F32R)
            nc.sync.dma_start(ones_row[:], onesr_d[:])

            qw_sb = pers.tile([128, CT, 3 * C], F32R)
            for ci in range(CT):
                d = nc.sync.dma_start(qw_sb[:, ci, :], qw_d[:, ci, :])
                add_dep_helper(d.ins, x_dmas[-1].ins, sync=True,
                               reason="let x DMA finish first")
            pw_sb = pers.tile([128, CT, C], F32R)
            d = nc.sync.dma_start(pw_sb[:], pw_d[:])
            add_dep_helper(d.ins, x_dmas[-1].ins, sync=True,
                           reason="let x DMA finish first")

            eps_sb = pers.tile([G, 1], F32)
            nc.vector.memset(eps_sb[:], EPS)
            ones_row32 = pers.tile([1, 128], F32)
            nc.vector.memset(ones_row32[:], 1.0)

            warm_ps = psp.tile([128, 512], F32, tag="ps")
            for _ in range(38):
                nc.tensor.matmul(
                    warm_ps[:], onesc_sb[:, 0:128], onesc_sb[:], start=True, stop=True
                )

            statcols = pers.tile([128, CT, 2], F32)
            for ct in range(CT):
                st6 = spool.tile([128, 2, 6], F32, tag="st6")
                nc.vector.bn_stats(st6[:, 0, :], x_sb[:, ct, 0:512])
                nc.vector.bn_stats(st6[:, 1, :], x_sb[:, ct, 512:1024])
                mv = spool.tile([128, 2], F32, tag="mv")
                nc.vector.bn_aggr(mv[:], st6[:])
                nc.vector.tensor_copy(statcols[:, ct, 0:1], mv[:, 0:1])
                nc.vector.tensor_mul(statcols[:, ct, 1:2], mv[:, 0:1], mv[:, 0:1])
                nc.vector.tensor_add(
                    statcols[:, ct, 1:2], statcols[:, ct, 1:2], mv[:, 1:2]
                )

            gsum_ps = psp.tile([G, 2], F32, tag="ps")
            for ct in range(CT):
                nc.tensor.matmul(
                    gsum_ps[:],
                    ind1_sb[:, ct * G : (ct + 1) * G],
                    statcols[:, ct, :],
                    start=(ct == 0),
                    stop=(ct == CT - 1),
                )
            gs_sb = spool.tile([G, 2], F32, tag="gs")
            nc.vector.tensor_scalar_mul(gs_sb[:], gsum_ps[:], 1.0 / GS)
            var32 = spool.tile([G, 1], F32, tag="var32")
            nc.vector.tensor_mul(var32[:], gs_sb[:, 0:1], gs_sb[:, 0:1])
            nc.vector.tensor_sub(var32[:], gs_sb[:, 1:2], var32[:])
            grow = pers.tile([G, 2], F32)
            lnv = spool.tile([G, 1], F32, tag="lnv")
            nc.scalar.activation(
                lnv[:], var32[:], mybir.ActivationFunctionType.Ln, bias=eps_sb[:]
            )
            nc.scalar.activation(
                grow[:, 0:1], lnv[:], mybir.ActivationFunctionType.Exp, scale=-0.5
            )
            nc.vector.tensor_mul(grow[:, 1:2], gs_sb[:, 0:1], grow[:, 0:1])

            xn_sb = pers.tile([128, CT, NPIX], F32R)
            chsb = pers.tile([128, CT, 2], F32)
            for ct in range(CT):
                bc_ps = psp.tile([128, 2], F32, tag="ps")
                nc.tensor.matmul(
                    bc_ps[:],
                    ind2_sb[:, ct * 128 : (ct + 1) * 128],
                    grow[:],
                    start=True,
                    stop=True,
                )
                nc.vector.tensor_mul(
                    chsb[:, ct, 0:1], gnw_sb[:, ct : ct + 1], bc_ps[:, 0:1]
                )
                nc.vector.tensor_mul(
                    chsb[:, ct, 1:2], gnw_sb[:, ct : ct + 1], bc_ps[:, 1:2]
                )
                nc.vector.tensor_sub(
                    chsb[:, ct, 1:2], gnb_sb[:, ct : ct + 1], chsb[:, ct, 1:2]
                )
                nc.vector.tensor_scalar(
                    out=xn_sb[:, ct, :],
                    in0=x_sb[:, ct, :],
                    scalar1=chsb[:, ct, 0:1],
                    scalar2=chsb[:, ct, 1:2],
                    op0=mybir.AluOpType.mult,
                    op1=mybir.AluOpType.add,
                )

            q_sb = pers.tile([128, CT, NPIX], F32R)
            k_sb = pers.tile([128, CT, NPIX], F32R)
            for dst, wofs, b_sb in ((q_sb, 0, qb_sb), (k_sb, C, kb_sb)):
                for co in range(CT):
                    for nh in range(NH):
                        ps = psp.tile([128, 512], F32, tag="ps")
                        for ci in range(CT):
                            nc.tensor.matmul(
                                ps[:],
                                qw_sb[:, ci, wofs + co * 128 : wofs + (co + 1) * 128],
                                xn_sb[:, ci, nh * 512 : (nh + 1) * 512],
                                start=(ci == 0),
                                stop=(ci == CT - 1),
                            )
                        nc.scalar.activation(
                            dst[:, co, nh * 512 : (nh + 1) * 512],
                            ps[:],
                            mybir.ActivationFunctionType.Identity,
                            bias=b_sb[:, co : co + 1],
                        )

            vt_sb = pers.tile([128, JT, C], F32R)
            for jt in range(JT):
                ps = psp.tile([128, 512], F32, tag="ps")
                for ci in range(CT):
                    nc.tensor.matmul(
                        ps[:],
                        xn_sb[:, ci, jt * 128 : (jt + 1) * 128],
                        qw_sb[:, ci, 2 * C : 3 * C],
                        start=(ci == 0),
                        stop=(ci == CT - 1),
                    )
                nc.vector.tensor_copy(vt_sb[:, jt, :], ps[:])

            e_sb = pers.tile([128, JT, NPIX], F32R)
            recip_sb = pers.tile([1, NPIX], F32)
            for nh in range(NH):
                dps = psp.tile([1, 512], F32, name=f"den{nh}", tag="ps")
                for jt in range(JT):
                    ps = psp.tile([128, 512], F32, tag="ps")
                    for ci in range(CT):
                        nc.tensor.matmul(
                            ps[:],
                            k_sb[:, ci, jt * 128 : (jt + 1) * 128],
                            q_sb[:, ci, nh * 512 : (nh + 1) * 512],
                            start=(ci == 0),
                            stop=(ci == CT - 1),
                        )
                    esl = e_sb[:, jt, nh * 512 : (nh + 1) * 512]
                    nc.scalar.activation(
                        esl, ps[:], mybir.ActivationFunctionType.Exp, scale=SCALE
                    )
                    nc.tensor.matmul(
                        dps[:],
                        onesc_sb[:, 0:1],
                        esl,
                        start=(jt == 0),
                        stop=(jt == JT - 1),
                    )
                rsl = recip_sb[0:1, nh * 512 : (nh + 1) * 512]
                rscr = spool.tile([1, 512], F32, tag="rscr")
                nc.vector.reciprocal_approx_accurate(rsl, dps[:], rscr[:])

            rb_sb = pers.tile([128, NPIX], F32)
            att_sb = pers.tile([128, CT, NPIX], F32R)
            for nh in range(NH):
                bp = psp.tile([128, 512], F32, name=f"bp{nh}", tag="ps")
                nc.tensor.matmul(
                    bp[:],
                    ones_row32[0:1, :],
                    recip_sb[0:1, nh * 512 : (nh + 1) * 512],
                    start=True,
                    stop=True,
                )
                nc.scalar.activation(
                    rb_sb[:, nh * 512 : (nh + 1) * 512],
                    bp[:],
                    mybir.ActivationFunctionType.Identity,
                )
                for ct in range(CT):
                    ps = psp.tile([128, 512], F32, tag="ps")
                    for jt in range(JT):
                        nc.tensor.matmul(
                            ps[:],
                            vt_sb[:, jt, ct * 128 : (ct + 1) * 128],
                            e_sb[:, jt, nh * 512 : (nh + 1) * 512],
                            start=(jt == 0),
                            stop=(jt == JT - 1),
                        )
                    nc.vector.tensor_mul(
                        att_sb[:, ct, nh * 512 : (nh + 1) * 512],
                        ps[:],
                        rb_sb[:, nh * 512 : (nh + 1) * 512],
                    )

            for nh in range(NH):
                for co in range(CT):
                    ps = psp.tile([128, 512], F32, tag="ps")
                    for ci in range(CT):
                        nc.tensor.matmul(
                            ps[:],
                            pw_sb[:, ci, co * 128 : (co + 1) * 128],
                            att_sb[:, ci, nh * 512 : (nh + 1) * 512],
                            start=(ci == 0),
                            stop=(ci == CT - 1),
                        )
                    sl = (slice(None), co, slice(nh * 512, (nh + 1) * 512))
                    nc.vector.scalar_tensor_tensor(
                        out=x_sb[sl],
                        in0=ps[:],
                        scalar=pb_sb[:, co : co + 1],
                        in1=x_sb[sl],
                        op0=mybir.AluOpType.add,
                        op1=mybir.AluOpType.add,
                    )
                    nc.sync.dma_start(y_d[sl], x_sb[sl])

    nc.compile()
    return nc


def kernel(x, gn_weight, gn_bias, qkv_w, qkv_b, proj_w, proj_b):
    global LAST_RESULTS
    b, c, h, w = x.shape
    assert (b, c, h * w) == (8, C, NPIX)

    qkv_b = np.asarray(qkv_b, np.float32)
    qkv_w = np.asarray(qkv_w, np.float32)
    proj_w = np.asarray(proj_w, np.float32)
    # The per-query bias term cancels in softmax; a nonzero q-bias would
    # contribute a per-key term, so only then fall back to explicit q/k.
    fold_qk = not np.any(qkv_b[0:C])
    # fp8 build assumes gn weight==1 / bias==0 (xn scale/bias come straight
    # from the group broadcast).
    trivial_gn = (
        not np.any(np.asarray(gn_weight, np.float32) != 1.0)
        and not np.any(np.asarray(gn_bias, np.float32))
    )
    use_fp8 = fold_qk and trivial_gn

    key = ("nc", use_fp8)
    if key not in _cache:
        _cache[key] = _build_fp8() if use_fp8 else _build_f32r()
    nc = _cache[key]

    def col(v):  # [512] vector -> [128, CT] per-partition columns
        return np.ascontiguousarray(np.asarray(v, np.float32).reshape(CT, 128).T)

    def wtile(wT, cols, dt=np.float32):  # [c_in, cols] -> [128, CT, cols]
        return np.ascontiguousarray(
            np.asarray(wT).astype(dt).reshape(CT, 128, cols).transpose(1, 0, 2)
        )

    if use_fp8:
        # A^T = W_q^T W_k in fp64 (so that lhsT-layout gives t = W_k^T W_q xn),
        # scaled x16 to keep e4m3 operands in the normal range.
        At = (qkv_w[0:C].astype(np.float64).T @ qkv_w[C : 2 * C].astype(np.float64))
        shared = {
            "qa": wtile(WSCALE * At, C, ml_dtypes.float8_e4m3fn),
            "qv": wtile(WSCALE * qkv_w[2 * C :].T.astype(np.float64), C,
                        ml_dtypes.float8_e4m3fn),
            "pw": wtile(proj_w.T, C, ml_dtypes.float8_e4m3fn),
            # attention rows sum to 1, so att(v + b_v) = att(v) + b_v; fold the
            # v bias through proj into the proj bias on the host.
            "pb": col(proj_b + proj_w @ qkv_b[2 * C :]),
            "ones8": np.full((128, 2, 128), WSCALE, ml_dtypes.float8_e4m3fn),
            "onesr": np.ones((1, 128), ml_dtypes.bfloat16),
        }
    else:
        shared = {
            "qw": wtile(qkv_w.T, 3 * C),
            "pw": wtile(proj_w.T, C),
            "gnw": col(gn_weight),
            "gnb": col(gn_bias),
            "pb": col(proj_b + proj_w @ qkv_b[2 * C :]),
            "qb": col(qkv_b[0:C]),
            "kb": col(qkv_b[C : 2 * C]),
            "onesc": np.ones((128, 512), np.float32),
            "onesr": np.ones((1, 128), np.float32),
        }

    xs = np.asarray(x, np.float32).reshape(b, CT, 128, NPIX)
    in_maps = [
        {"x": np.ascontiguousarray(xs[i].transpose(1, 0, 2)), **shared}
        for i in range(b)
    ]

    res = run_bass_kernel_spmd(
        nc, in_maps, core_ids=list(range(8)), trace=TRACE, **TRACE_KW
    )
    LAST_RESULTS = res
    out = np.stack(
        [r["y"].transpose(1, 0, 2).reshape(c, h, w) for r in res.results]
    )
    return out.astype(np.float32)
